# revision 8
# baseline (speedup 1.0000x reference)
"""CAM_Module (channel attention) Trainium2 Bass kernel.

x: (16, 512, 64, 64) f32, gamma: (1,) f32
  xf = x.reshape(B, C, N)           N = 4096
  energy = xf @ xf^T                (B, C, C)
  att = softmax(max(energy) - energy, axis=-1)   == softmax(-energy) (shift-invariant)
  out = gamma * (att @ xf) + x

Sharding: data-parallel over batch, 2 batches per core on 8 cores.

Per-core pipeline (per batch):
  - SWDGE cast-load x -> SBUF as f32r (rounded fp32, full-rate matmul dtype)
  - PE transpose-mode: xf^T chunks (n on partitions) -> PSUM -> ACT copy -> SBUF
  - MM1 (f32r): energy[i, :] accumulated over 32 k-chunks into 4 PSUM banks
  - softmax: DVE row-min, ACT exp(min - e) with row-sum accumulation,
    DVE reciprocal, scale by gamma/Z
  - PE transpose att -> att^T
  - MM2 (f32r): att^T.T @ xf accumulated over 4 j-chunks, DVE adds residual x
  - HWDGE store
"""

import sys

if "/opt/trn_rl_repo" not in sys.path:
    sys.path.insert(0, "/opt/trn_rl_repo")

from contextlib import ExitStack

import numpy as np

import concourse.bass as bass
import concourse.tile as tile
from concourse import bacc, mybir
from concourse.bass_utils import run_bass_kernel_spmd
from concourse.masks import make_identity

N_CORES = 8
B, C, H, W = 16, 512, 64, 64
N = H * W                    # 4096
BPC = B // N_CORES           # batches per core = 2
CT = C // 128                # 4 c-tiles
KT = N // 128                # 32 k-chunks (transposed layout)
NCH = N // 512               # 8 moving chunks for MM2

F32 = mybir.dt.float32
F32R = mybir.dt.float32r


def _build_nc():
    nc = bacc.Bacc("TRN2", target_bir_lowering=False, debug=False,
                   num_devices=N_CORES)
    x_d = nc.dram_tensor("x", [BPC, C, N], F32, kind="ExternalInput").ap()
    g_d = nc.dram_tensor("gamma", [1], F32, kind="ExternalInput").ap()
    o_d = nc.dram_tensor("out", [BPC, C, N], F32, kind="ExternalOutput").ap()

    with tile.TileContext(nc) as tc, ExitStack() as ctx:
        xf_pool = ctx.enter_context(tc.tile_pool(name="xf", bufs=BPC * CT))
        xfT_pool = ctx.enter_context(tc.tile_pool(name="xfT", bufs=6))
        s_pool = ctx.enter_context(tc.tile_pool(name="s", bufs=CT))
        att_pool = ctx.enter_context(tc.tile_pool(name="att", bufs=CT))
        attT_pool = ctx.enter_context(tc.tile_pool(name="attT", bufs=CT))
        out_pool = ctx.enter_context(tc.tile_pool(name="outp", bufs=2))
        stat_pool = ctx.enter_context(tc.tile_pool(name="stat", bufs=4 * CT))
        one_pool = ctx.enter_context(tc.tile_pool(name="one", bufs=1))
        pT = ctx.enter_context(tc.tile_pool(name="pT", bufs=2, space="PSUM"))
        pE = ctx.enter_context(tc.tile_pool(name="pE", bufs=CT, space="PSUM"))
        pO = ctx.enter_context(tc.tile_pool(name="pO", bufs=2, space="PSUM"))

        # identity for PE transpose-mode (f32r so dtypes match the data)
        ident_f = one_pool.tile([128, 128], F32, tag="idf")
        make_identity(nc, ident_f[:])
        ident = one_pool.tile([128, 128], F32R, tag="idr")
        nc.vector.tensor_copy(ident[:], ident_f[:])

        # broadcast gamma to all 128 partitions via K=1 matmul with ones
        g_sb = one_pool.tile([1, 1], F32, tag="gsb")
        nc.sync.dma_start(g_sb[:], g_d.rearrange("(a b) -> a b", a=1))
        ones = one_pool.tile([1, 128], F32, tag="ones")
        nc.vector.memset(ones[:], 1.0)
        pG = pT.tile([128, 1], F32, tag="pt", name="pG")
        nc.tensor.matmul(pG[:], ones[:], g_sb[:], start=True, stop=True)
        g_bc = one_pool.tile([128, 1], F32, tag="gbc")
        nc.vector.tensor_copy(g_bc[:], pG[:])

        for b in range(BPC):
            # ---- load x (cast f32 -> f32r during DMA) ----
            xf = []
            for ct in range(CT):
                t = xf_pool.tile([128, N], F32R, tag="xf")
                nc.gpsimd.dma_start(t[:], x_d[b, ct * 128:(ct + 1) * 128, :])
                xf.append(t)

            # ---- transpose to (n, c) layout + Gram matmuls ----
            e_ps = [
                pE.tile([128, C], F32, tag="pe", name=f"pe_{b}_{i}")
                for i in range(CT)
            ]
            for k in range(KT):
                tp = pT.tile([128, C], F32R, tag="pt")
                for ct in range(CT):
                    nc.tensor.transpose(
                        tp[:, ct * 128:(ct + 1) * 128],
                        xf[ct][:, k * 128:(k + 1) * 128],
                        ident[:],
                    )
                xT = xfT_pool.tile([128, C], F32R, tag="xT")
                nc.scalar.copy(xT[:], tp[:])
                for it in range(CT):
                    nc.tensor.matmul(
                        e_ps[it][:],
                        xT[:, it * 128:(it + 1) * 128],
                        xT[:],
                        start=(k == 0),
                        stop=(k == KT - 1),
                    )

            # ---- softmax(-energy) rows, scaled by gamma ----
            att = []
            for it in range(CT):
                m = stat_pool.tile([128, 1], F32, tag="m")
                nc.vector.tensor_reduce(
                    m[:], e_ps[it][:], axis=mybir.AxisListType.X,
                    op=mybir.AluOpType.min,
                )
                s = s_pool.tile([128, C], F32, tag="s")
                z = stat_pool.tile([128, 1], F32, tag="z")
                nc.scalar.activation(
                    s[:], e_ps[it][:], mybir.ActivationFunctionType.Exp,
                    bias=m[:], scale=-1.0, accum_out=z[:],
                )
                rz = stat_pool.tile([128, 1], F32, tag="rz")
                nc.vector.reciprocal(rz[:], z[:])
                g = stat_pool.tile([128, 1], F32, tag="g")
                nc.vector.tensor_mul(g[:], rz[:], g_bc[:])
                a = att_pool.tile([128, C], F32R, tag="a")
                nc.vector.tensor_scalar_mul(a[:], s[:], g[:])
                att.append(a)

            # ---- transpose attention ----
            attT = []
            for jt in range(CT):
                tp = pT.tile([128, C], F32R, tag="pt")
                for it in range(CT):
                    nc.tensor.transpose(
                        tp[:, it * 128:(it + 1) * 128],
                        att[it][:, jt * 128:(jt + 1) * 128],
                        ident[:],
                    )
                aT = attT_pool.tile([128, C], F32R, tag="aT")
                nc.vector.tensor_copy(aT[:], tp[:])
                attT.append(aT)

            # ---- out = att @ xf + x ----
            for it in range(CT):
                o = out_pool.tile([128, N], F32, tag="o")
                for nch in range(NCH):
                    po = pO.tile([128, 512], F32, tag="po")
                    for jt in range(CT):
                        nc.tensor.matmul(
                            po[:],
                            attT[jt][:, it * 128:(it + 1) * 128],
                            xf[jt][:, nch * 512:(nch + 1) * 512],
                            start=(jt == 0),
                            stop=(jt == CT - 1),
                        )
                    nc.vector.tensor_add(
                        o[:, nch * 512:(nch + 1) * 512], po[:],
                        xf[it][:, nch * 512:(nch + 1) * 512],
                    )
                nc.sync.dma_start(o_d[b, it * 128:(it + 1) * 128, :], o[:])

    nc.compile()
    return nc


_RUNNER = None


def _build_runner():
    """Compile once; return a callable (xf_full, gamma) -> out_full.

    Mirrors concourse.bass2jax.run_bass_via_pjrt but caches the jitted
    shard_map executable so repeated kernel() calls don't re-lower.
    """
    import jax
    from jax.sharding import Mesh, PartitionSpec
    from jax.experimental.shard_map import shard_map

    from concourse import bass2jax, mybir as _mybir
    from concourse.bass2jax import _bass_exec_p, partition_id_tensor

    nc = _build_nc()
    bass2jax.install_neuronx_cc_hook()

    partition_name = (
        nc.partition_id_tensor.name if nc.partition_id_tensor else None
    )
    in_names, out_names, out_avals, zero_shapes = [], [], [], []
    for alloc in nc.m.functions[0].allocations:
        if not isinstance(alloc, _mybir.MemoryLocationSet):
            continue
        name = alloc.memorylocations[0].name
        if alloc.kind == "ExternalInput":
            if name != partition_name:
                in_names.append(name)
        elif alloc.kind == "ExternalOutput":
            shape = tuple(alloc.tensor_shape)
            dtype = _mybir.dt.np(alloc.dtype)
            out_names.append(name)
            out_avals.append(jax.core.ShapedArray(shape, dtype))
            zero_shapes.append((shape, dtype))
    n_params = len(in_names)
    all_names = list(in_names) + list(out_names)
    if partition_name is not None:
        all_names.append(partition_name)
    donate = tuple(range(n_params, n_params + len(out_names)))

    def _body(*args):
        operands = list(args)
        if partition_name is not None:
            operands.append(partition_id_tensor())
        return tuple(
            _bass_exec_p.bind(
                *operands,
                out_avals=tuple(out_avals),
                in_names=tuple(all_names),
                out_names=tuple(out_names),
                lowering_input_output_aliases=(),
                sim_require_finite=True,
                sim_require_nnan=True,
                nc=nc,
            )
        )

    devices = jax.devices()[:N_CORES]
    mesh = Mesh(np.asarray(devices), ("core",))
    n_in = n_params + len(out_names)
    sharded = jax.jit(
        shard_map(
            _body,
            mesh=mesh,
            in_specs=(PartitionSpec("core"),) * n_in,
            out_specs=(PartitionSpec("core"),) * len(out_names),
            check_rep=False,
        ),
        donate_argnums=donate,
        keep_unused=True,
    )

    # in_names order is discovered from allocations; map our two inputs
    assert set(in_names) == {"x", "gamma"}, in_names

    def run(xf_full, gamma):
        per_in = {
            "x": xf_full,  # (16, 512, 4096) == concat of per-core (2, 512, 4096)
            "gamma": np.ascontiguousarray(
                np.broadcast_to(gamma.reshape(1), (N_CORES,))
            ),
        }
        concat_in = [per_in[name] for name in in_names]
        zeros = [
            np.zeros((N_CORES * s[0],) + s[1:], d) for s, d in zero_shapes
        ]
        out_arrs = sharded(*concat_in, *zeros)
        return np.asarray(out_arrs[out_names.index("out")])

    return run


def _get_runner():
    global _RUNNER
    if _RUNNER is None:
        _RUNNER = _build_runner()
    return _RUNNER


def kernel(x, gamma):
    assert x.shape == (B, C, H, W)
    run = _get_runner()
    xf = np.ascontiguousarray(np.asarray(x, np.float32).reshape(B, C, N))
    g = np.asarray(gamma, np.float32)
    out = run(xf, g)
    return out.reshape(B, C, H, W).astype(np.float32, copy=False)


# revision 12
# speedup vs baseline: 3058.2356x; 3058.2356x over previous
"""CAM_Module (channel attention) Trainium2 Bass kernel.

x: (16, 512, 64, 64) f32, gamma: (1,) f32
  xf = x.reshape(B, C, N)           N = 4096
  energy = xf @ xf^T                (B, C, C)
  att = softmax(max(energy) - energy, axis=-1)   == softmax(-energy) (shift-invariant)
  out = gamma * (att @ xf) + x

Sharding: data-parallel over batch, 2 batches per core on 8 cores.

Per-core pipeline (per batch):
  - SWDGE cast-load x -> SBUF as f32r (rounded fp32, full-rate matmul dtype)
  - PE transpose-mode: xf^T chunks (n on partitions) -> PSUM -> ACT copy -> SBUF
  - MM1 (f32r): energy[i, :] accumulated over 32 k-chunks into 4 PSUM banks
  - softmax: DVE row-min, ACT exp(min - e) with row-sum accumulation,
    DVE reciprocal, scale by gamma/Z
  - PE transpose att -> att^T
  - MM2 (f32r): att^T.T @ xf accumulated over 4 j-chunks, DVE adds residual x
  - HWDGE store
"""

import sys

if "/opt/trn_rl_repo" not in sys.path:
    sys.path.insert(0, "/opt/trn_rl_repo")

from contextlib import ExitStack

import numpy as np

import concourse.bass as bass
import concourse.tile as tile
from concourse import bacc, mybir
from concourse.bass_utils import run_bass_kernel_spmd
from concourse.masks import make_identity

N_CORES = 8
B, C, H, W = 16, 512, 64, 64
N = H * W                    # 4096
BPC = B // N_CORES           # batches per core = 2
CT = C // 128                # 4 c-tiles
KT = N // 128                # 32 k-chunks (transposed layout)
NCH = N // 512               # 8 moving chunks for MM2

F32 = mybir.dt.float32
F32R = mybir.dt.float32r


def _build_nc(reps=1):
    nc = bacc.Bacc("TRN2", target_bir_lowering=False, debug=False,
                   num_devices=N_CORES)
    x_d = nc.dram_tensor("x", [BPC, C, N], F32, kind="ExternalInput").ap()
    g_d = nc.dram_tensor("gamma", [1], F32, kind="ExternalInput").ap()
    o_d = nc.dram_tensor("out", [BPC, C, N], F32, kind="ExternalOutput").ap()

    with tile.TileContext(nc) as tc, ExitStack() as ctx:
        xf_pool = ctx.enter_context(tc.tile_pool(name="xf", bufs=BPC * CT))
        xfT_pool = ctx.enter_context(tc.tile_pool(name="xfT", bufs=6))
        s_pool = ctx.enter_context(tc.tile_pool(name="s", bufs=CT))
        att_pool = ctx.enter_context(tc.tile_pool(name="att", bufs=CT))
        attT_pool = ctx.enter_context(tc.tile_pool(name="attT", bufs=CT))
        out_pool = ctx.enter_context(tc.tile_pool(name="outp", bufs=2))
        stat_pool = ctx.enter_context(tc.tile_pool(name="stat", bufs=4 * CT))
        one_pool = ctx.enter_context(tc.tile_pool(name="one", bufs=1))
        pT = ctx.enter_context(tc.tile_pool(name="pT", bufs=2, space="PSUM"))
        pE = ctx.enter_context(tc.tile_pool(name="pE", bufs=CT, space="PSUM"))
        pO = ctx.enter_context(tc.tile_pool(name="pO", bufs=2, space="PSUM"))

        # identity for PE transpose-mode (f32r so dtypes match the data)
        ident_f = one_pool.tile([128, 128], F32, tag="idf")
        make_identity(nc, ident_f[:])
        ident = one_pool.tile([128, 128], F32R, tag="idr")
        nc.vector.tensor_copy(ident[:], ident_f[:])

        # broadcast gamma to all 128 partitions via K=1 matmul with ones
        g_sb = one_pool.tile([1, 1], F32, tag="gsb")
        nc.sync.dma_start(g_sb[:], g_d.rearrange("(a b) -> a b", a=1))
        ones = one_pool.tile([1, 128], F32, tag="ones")
        nc.vector.memset(ones[:], 1.0)
        pG = pT.tile([128, 1], F32, tag="pt", name="pG")
        nc.tensor.matmul(pG[:], ones[:], g_sb[:], start=True, stop=True)
        g_bc = one_pool.tile([128, 1], F32, tag="gbc")
        nc.vector.tensor_copy(g_bc[:], pG[:])

        loop_ctx = tc.For_i(0, reps, 1) if reps > 1 else None
        if loop_ctx is not None:
            ctx.enter_context(loop_ctx)
        for b in range(BPC):
            # ---- load x (cast f32 -> f32r during DMA) ----
            xf = []
            for ct in range(CT):
                t = xf_pool.tile([128, N], F32R, tag="xf")
                nc.gpsimd.dma_start(t[:], x_d[b, ct * 128:(ct + 1) * 128, :])
                xf.append(t)

            # ---- transpose to (n, c) layout + Gram matmuls ----
            e_ps = [
                pE.tile([128, C], F32, tag="pe", name=f"pe_{b}_{i}")
                for i in range(CT)
            ]
            for k in range(KT):
                tp = pT.tile([128, C], F32R, tag="pt")
                for ct in range(CT):
                    nc.tensor.transpose(
                        tp[:, ct * 128:(ct + 1) * 128],
                        xf[ct][:, k * 128:(k + 1) * 128],
                        ident[:],
                    )
                xT = xfT_pool.tile([128, C], F32R, tag="xT")
                nc.scalar.copy(xT[:], tp[:])
                for it in range(CT):
                    nc.tensor.matmul(
                        e_ps[it][:],
                        xT[:, it * 128:(it + 1) * 128],
                        xT[:],
                        start=(k == 0),
                        stop=(k == KT - 1),
                    )

            # ---- softmax(-energy) rows, scaled by gamma ----
            att = []
            for it in range(CT):
                m = stat_pool.tile([128, 1], F32, tag="m")
                nc.vector.tensor_reduce(
                    m[:], e_ps[it][:], axis=mybir.AxisListType.X,
                    op=mybir.AluOpType.min,
                )
                s = s_pool.tile([128, C], F32, tag="s")
                z = stat_pool.tile([128, 1], F32, tag="z")
                nc.scalar.activation(
                    s[:], e_ps[it][:], mybir.ActivationFunctionType.Exp,
                    bias=m[:], scale=-1.0, accum_out=z[:],
                )
                rz = stat_pool.tile([128, 1], F32, tag="rz")
                nc.vector.reciprocal(rz[:], z[:])
                g = stat_pool.tile([128, 1], F32, tag="g")
                nc.vector.tensor_mul(g[:], rz[:], g_bc[:])
                a = att_pool.tile([128, C], F32R, tag="a")
                nc.vector.tensor_scalar_mul(a[:], s[:], g[:])
                att.append(a)

            # ---- transpose attention ----
            attT = []
            for jt in range(CT):
                tp = pT.tile([128, C], F32R, tag="pt")
                for it in range(CT):
                    nc.tensor.transpose(
                        tp[:, it * 128:(it + 1) * 128],
                        att[it][:, jt * 128:(jt + 1) * 128],
                        ident[:],
                    )
                aT = attT_pool.tile([128, C], F32R, tag="aT")
                nc.vector.tensor_copy(aT[:], tp[:])
                attT.append(aT)

            # ---- out = att @ xf + x ----
            for it in range(CT):
                o = out_pool.tile([128, N], F32, tag="o")
                for nch in range(NCH):
                    po = pO.tile([128, 512], F32, tag="po")
                    for jt in range(CT):
                        nc.tensor.matmul(
                            po[:],
                            attT[jt][:, it * 128:(it + 1) * 128],
                            xf[jt][:, nch * 512:(nch + 1) * 512],
                            start=(jt == 0),
                            stop=(jt == CT - 1),
                        )
                    nc.vector.tensor_add(
                        o[:, nch * 512:(nch + 1) * 512], po[:],
                        xf[it][:, nch * 512:(nch + 1) * 512],
                    )
                nc.sync.dma_start(o_d[b, it * 128:(it + 1) * 128, :], o[:])

    nc.compile()
    return nc


_RUNNER = None


def _build_runner(nc=None):
    """Compile once; return a callable (xf_full, gamma) -> out_full.

    Mirrors concourse.bass2jax.run_bass_via_pjrt but caches the jitted
    shard_map executable so repeated kernel() calls don't re-lower, and
    keeps the output-seed zero buffers resident on device.
    """
    import jax
    from jax.sharding import Mesh, NamedSharding, PartitionSpec
    from jax.experimental.shard_map import shard_map

    from concourse import bass2jax, mybir as _mybir
    from concourse.bass2jax import _bass_exec_p, partition_id_tensor

    if nc is None:
        nc = _build_nc()
    bass2jax.install_neuronx_cc_hook()

    partition_name = (
        nc.partition_id_tensor.name if nc.partition_id_tensor else None
    )
    in_names, out_names, out_avals, zero_shapes = [], [], [], []
    for alloc in nc.m.functions[0].allocations:
        if not isinstance(alloc, _mybir.MemoryLocationSet):
            continue
        name = alloc.memorylocations[0].name
        if alloc.kind == "ExternalInput":
            if name != partition_name:
                in_names.append(name)
        elif alloc.kind == "ExternalOutput":
            shape = tuple(alloc.tensor_shape)
            dtype = _mybir.dt.np(alloc.dtype)
            out_names.append(name)
            out_avals.append(jax.core.ShapedArray(shape, dtype))
            zero_shapes.append((shape, dtype))
    n_params = len(in_names)
    all_names = list(in_names) + list(out_names)
    if partition_name is not None:
        all_names.append(partition_name)
    donate = tuple(range(n_params, n_params + len(out_names)))

    def _body(*args):
        operands = list(args)
        if partition_name is not None:
            operands.append(partition_id_tensor())
        return tuple(
            _bass_exec_p.bind(
                *operands,
                out_avals=tuple(out_avals),
                in_names=tuple(all_names),
                out_names=tuple(out_names),
                lowering_input_output_aliases=(),
                sim_require_finite=True,
                sim_require_nnan=True,
                nc=nc,
            )
        )

    devices = jax.devices()[:N_CORES]
    mesh = Mesh(np.asarray(devices), ("core",))
    n_in = n_params + len(out_names)
    sharded = jax.jit(
        shard_map(
            _body,
            mesh=mesh,
            in_specs=(PartitionSpec("core"),) * n_in,
            out_specs=(PartitionSpec("core"),) * len(out_names),
            check_rep=False,
        ),
        keep_unused=True,
    )

    # in_names order is discovered from allocations; map our two inputs
    assert set(in_names) == {"x", "gamma"}, in_names

    # output-seed buffers created on device once (kernel writes out fully)
    sh = NamedSharding(mesh, PartitionSpec("core"))
    zeros_dev = [
        jax.jit(
            lambda s=s, d=d: jax.numpy.zeros((N_CORES * s[0],) + s[1:], d),
            out_shardings=sh,
        )()
        for s, d in zero_shapes
    ]
    jax.block_until_ready(zeros_dev)

    def run(xf_full, gamma):
        per_in = {
            "x": xf_full,  # (16, 512, 4096) == concat of per-core (2, 512, 4096)
            "gamma": np.ascontiguousarray(
                np.broadcast_to(np.asarray(gamma, np.float32).reshape(1),
                                (N_CORES,))
            ),
        }
        concat_in = [per_in[name] for name in in_names]
        out_arrs = sharded(*concat_in, *zeros_dev)
        return np.asarray(out_arrs[out_names.index("out")])

    return run


def _get_runner():
    global _RUNNER
    if _RUNNER is None:
        _RUNNER = _build_runner()
    return _RUNNER


def kernel(x, gamma):
    assert x.shape == (B, C, H, W)
    run = _get_runner()
    xf = np.ascontiguousarray(np.asarray(x, np.float32).reshape(B, C, N))
    g = np.asarray(gamma, np.float32)
    out = run(xf, g)
    return out.reshape(B, C, H, W).astype(np.float32, copy=False)


# revision 13
# speedup vs baseline: 36187.9715x; 11.8330x over previous
"""CAM_Module (channel attention) Trainium2 Bass kernel.

x: (16, 512, 64, 64) f32, gamma: (1,) f32
  xf = x.reshape(B, C, N)           N = 4096
  energy = xf @ xf^T                (B, C, C)
  att = softmax(max(energy) - energy, axis=-1)   == softmax(-energy) (shift-invariant)
  out = gamma * (att @ xf) + x

Sharding: data-parallel over batch, 2 batches per core on 8 cores.

Per-core pipeline (per batch):
  - SWDGE cast-load x -> SBUF as f32r (rounded fp32, full-rate matmul dtype)
  - PE transpose-mode: xf^T chunks (n on partitions) -> PSUM -> ACT copy -> SBUF
  - MM1 (f32r): energy[i, :] accumulated over 32 k-chunks into 4 PSUM banks
  - softmax: DVE row-min, ACT exp(min - e) with row-sum accumulation,
    DVE reciprocal, scale by gamma/Z
  - PE transpose att -> att^T
  - MM2 (f32r): att^T.T @ xf accumulated over 4 j-chunks, DVE adds residual x
  - HWDGE store
"""

import sys

if "/opt/trn_rl_repo" not in sys.path:
    sys.path.insert(0, "/opt/trn_rl_repo")

from contextlib import ExitStack

import numpy as np

import concourse.bass as bass
import concourse.tile as tile
from concourse import bacc, mybir
from concourse.bass_utils import run_bass_kernel_spmd
from concourse.masks import make_identity

N_CORES = 8
B, C, H, W = 16, 512, 64, 64
N = H * W                    # 4096
BPC = B // N_CORES           # batches per core = 2
CT = C // 128                # 4 c-tiles
KT = N // 128                # 32 k-chunks (transposed layout)
NCH = N // 512               # 8 moving chunks for MM2

F32 = mybir.dt.float32
F32R = mybir.dt.float32r


def _build_nc(reps=1):
    nc = bacc.Bacc("TRN2", target_bir_lowering=False, debug=False,
                   num_devices=N_CORES)
    x_d = nc.dram_tensor("x", [BPC, C, N], F32, kind="ExternalInput").ap()
    g_d = nc.dram_tensor("gamma", [1], F32, kind="ExternalInput").ap()
    o_d = nc.dram_tensor("out", [BPC, C, N], F32, kind="ExternalOutput").ap()

    with tile.TileContext(nc) as tc, ExitStack() as ctx:
        xf_pool = ctx.enter_context(tc.tile_pool(name="xf", bufs=BPC * CT))
        xfT_pool = ctx.enter_context(tc.tile_pool(name="xfT", bufs=6))
        s_pool = ctx.enter_context(tc.tile_pool(name="s", bufs=CT))
        att_pool = ctx.enter_context(tc.tile_pool(name="att", bufs=CT))
        attT_pool = ctx.enter_context(tc.tile_pool(name="attT", bufs=CT))
        out_pool = ctx.enter_context(tc.tile_pool(name="outp", bufs=2))
        stat_pool = ctx.enter_context(tc.tile_pool(name="stat", bufs=4 * CT))
        one_pool = ctx.enter_context(tc.tile_pool(name="one", bufs=1))
        pT = ctx.enter_context(tc.tile_pool(name="pT", bufs=2, space="PSUM"))
        pE = ctx.enter_context(tc.tile_pool(name="pE", bufs=CT, space="PSUM"))
        pO = ctx.enter_context(tc.tile_pool(name="pO", bufs=2, space="PSUM"))

        # identity for PE transpose-mode (f32r so dtypes match the data)
        ident_f = one_pool.tile([128, 128], F32, tag="idf")
        make_identity(nc, ident_f[:])
        ident = one_pool.tile([128, 128], F32R, tag="idr")
        nc.vector.tensor_copy(ident[:], ident_f[:])

        # broadcast gamma to all 128 partitions via K=1 matmul with ones
        g_sb = one_pool.tile([1, 1], F32, tag="gsb")
        nc.sync.dma_start(g_sb[:], g_d.rearrange("(a b) -> a b", a=1))
        ones = one_pool.tile([1, 128], F32, tag="ones")
        nc.vector.memset(ones[:], 1.0)
        pG = pT.tile([128, 1], F32, tag="pt", name="pG")
        nc.tensor.matmul(pG[:], ones[:], g_sb[:], start=True, stop=True)
        g_bc = one_pool.tile([128, 1], F32, tag="gbc")
        nc.vector.tensor_copy(g_bc[:], pG[:])

        loop_ctx = tc.For_i(0, reps, 1) if reps > 1 else None
        if loop_ctx is not None:
            ctx.enter_context(loop_ctx)
        for b in range(BPC):
            # ---- load x (cast f32 -> f32r during DMA) ----
            xf = []
            for ct in range(CT):
                t = xf_pool.tile([128, N], F32R, tag="xf")
                nc.gpsimd.dma_start(t[:], x_d[b, ct * 128:(ct + 1) * 128, :])
                xf.append(t)

            # ---- transpose to (n, c) layout + Gram matmuls ----
            e_ps = [
                pE.tile([128, C], F32, tag="pe", name=f"pe_{b}_{i}")
                for i in range(CT)
            ]
            for k in range(KT):
                tp = pT.tile([128, C], F32R, tag="pt")
                for ct in range(CT):
                    nc.tensor.transpose(
                        tp[:, ct * 128:(ct + 1) * 128],
                        xf[ct][:, k * 128:(k + 1) * 128],
                        ident[:],
                    )
                xT = xfT_pool.tile([128, C], F32R, tag="xT")
                nc.scalar.copy(xT[:], tp[:])
                for it in range(CT):
                    nc.tensor.matmul(
                        e_ps[it][:],
                        xT[:, it * 128:(it + 1) * 128],
                        xT[:],
                        start=(k == 0),
                        stop=(k == KT - 1),
                    )

            # ---- softmax(-energy) rows, scaled by gamma ----
            att = []
            for it in range(CT):
                m = stat_pool.tile([128, 1], F32, tag="m")
                nc.vector.tensor_reduce(
                    m[:], e_ps[it][:], axis=mybir.AxisListType.X,
                    op=mybir.AluOpType.min,
                )
                s = s_pool.tile([128, C], F32, tag="s")
                z = stat_pool.tile([128, 1], F32, tag="z")
                nc.scalar.activation(
                    s[:], e_ps[it][:], mybir.ActivationFunctionType.Exp,
                    bias=m[:], scale=-1.0, accum_out=z[:],
                )
                rz = stat_pool.tile([128, 1], F32, tag="rz")
                nc.vector.reciprocal(rz[:], z[:])
                g = stat_pool.tile([128, 1], F32, tag="g")
                nc.vector.tensor_mul(g[:], rz[:], g_bc[:])
                a = att_pool.tile([128, C], F32R, tag="a")
                nc.vector.tensor_scalar_mul(a[:], s[:], g[:])
                att.append(a)

            # ---- transpose attention ----
            attT = []
            for jt in range(CT):
                tp = pT.tile([128, C], F32R, tag="pt")
                for it in range(CT):
                    nc.tensor.transpose(
                        tp[:, it * 128:(it + 1) * 128],
                        att[it][:, jt * 128:(jt + 1) * 128],
                        ident[:],
                    )
                aT = attT_pool.tile([128, C], F32R, tag="aT")
                nc.vector.tensor_copy(aT[:], tp[:])
                attT.append(aT)

            # ---- out = att @ xf + x ----
            for it in range(CT):
                o = out_pool.tile([128, N], F32, tag="o")
                for nch in range(NCH):
                    po = pO.tile([128, 512], F32, tag="po")
                    for jt in range(CT):
                        nc.tensor.matmul(
                            po[:],
                            attT[jt][:, it * 128:(it + 1) * 128],
                            xf[jt][:, nch * 512:(nch + 1) * 512],
                            start=(jt == 0),
                            stop=(jt == CT - 1),
                        )
                    nc.vector.tensor_add(
                        o[:, nch * 512:(nch + 1) * 512], po[:],
                        xf[it][:, nch * 512:(nch + 1) * 512],
                    )
                nc.sync.dma_start(o_d[b, it * 128:(it + 1) * 128, :], o[:])

    nc.compile()
    return nc


_RUNNER = None


def _build_runner(nc=None):
    """Compile once; return a callable (xf_full, gamma) -> out_full.

    Mirrors concourse.bass2jax.run_bass_via_pjrt but caches the jitted
    shard_map executable so repeated kernel() calls don't re-lower, and
    keeps the output-seed zero buffers resident on device.
    """
    import jax
    from jax.sharding import Mesh, NamedSharding, PartitionSpec
    from jax.experimental.shard_map import shard_map

    from concourse import bass2jax, mybir as _mybir
    from concourse.bass2jax import _bass_exec_p, partition_id_tensor

    if nc is None:
        nc = _build_nc()
    bass2jax.install_neuronx_cc_hook()

    partition_name = (
        nc.partition_id_tensor.name if nc.partition_id_tensor else None
    )
    in_names, out_names, out_avals, zero_shapes = [], [], [], []
    for alloc in nc.m.functions[0].allocations:
        if not isinstance(alloc, _mybir.MemoryLocationSet):
            continue
        name = alloc.memorylocations[0].name
        if alloc.kind == "ExternalInput":
            if name != partition_name:
                in_names.append(name)
        elif alloc.kind == "ExternalOutput":
            shape = tuple(alloc.tensor_shape)
            dtype = _mybir.dt.np(alloc.dtype)
            out_names.append(name)
            out_avals.append(jax.core.ShapedArray(shape, dtype))
            zero_shapes.append((shape, dtype))
    n_params = len(in_names)
    all_names = list(in_names) + list(out_names)
    if partition_name is not None:
        all_names.append(partition_name)
    donate = tuple(range(n_params, n_params + len(out_names)))

    def _body(*args):
        operands = list(args)
        if partition_name is not None:
            operands.append(partition_id_tensor())
        return tuple(
            _bass_exec_p.bind(
                *operands,
                out_avals=tuple(out_avals),
                in_names=tuple(all_names),
                out_names=tuple(out_names),
                lowering_input_output_aliases=(),
                sim_require_finite=True,
                sim_require_nnan=True,
                nc=nc,
            )
        )

    devices = jax.devices()[:N_CORES]
    mesh = Mesh(np.asarray(devices), ("core",))
    n_in = n_params + len(out_names)
    sharded = jax.jit(
        shard_map(
            _body,
            mesh=mesh,
            in_specs=(PartitionSpec("core"),) * n_in,
            out_specs=(PartitionSpec("core"),) * len(out_names),
            check_rep=False,
        ),
        keep_unused=True,
    )

    # in_names order is discovered from allocations; map our two inputs
    assert set(in_names) == {"x", "gamma"}, in_names

    # output-seed buffers created on device once (kernel writes out fully)
    sh = NamedSharding(mesh, PartitionSpec("core"))
    zeros_dev = [
        jax.jit(
            lambda s=s, d=d: jax.numpy.zeros((N_CORES * s[0],) + s[1:], d),
            out_shardings=sh,
        )()
        for s, d in zero_shapes
    ]
    jax.block_until_ready(zeros_dev)

    def run(xf_full, gamma):
        per_in = {
            "x": xf_full,  # (16, 512, 4096) == concat of per-core (2, 512, 4096)
            "gamma": np.ascontiguousarray(
                np.broadcast_to(np.asarray(gamma, np.float32).reshape(1),
                                (N_CORES,))
            ),
        }
        concat_in = [per_in[name] for name in in_names]
        out_arrs = sharded(*concat_in, *zeros_dev)
        return np.asarray(out_arrs[out_names.index("out")])

    run.sharded = sharded
    run.zeros_dev = zeros_dev
    run.in_names = in_names
    run.out_names = out_names
    run.mesh = mesh
    return run


def _get_runner():
    global _RUNNER
    if _RUNNER is None:
        _RUNNER = _build_runner()
    return _RUNNER


def kernel(x, gamma):
    assert x.shape == (B, C, H, W)
    run = _get_runner()
    xf = np.ascontiguousarray(np.asarray(x, np.float32).reshape(B, C, N))
    g = np.asarray(gamma, np.float32)
    out = run(xf, g)
    return out.reshape(B, C, H, W).astype(np.float32, copy=False)


# revision 15
# speedup vs baseline: 36883.5308x; 1.0192x over previous
"""CAM_Module (channel attention) Trainium2 Bass kernel.

x: (16, 512, 64, 64) f32, gamma: (1,) f32
  xf = x.reshape(B, C, N)           N = 4096
  energy = xf @ xf^T                (B, C, C)
  att = softmax(max(energy) - energy, axis=-1)   == softmax(-energy) (shift-invariant)
  out = gamma * (att @ xf) + x

Sharding: data-parallel over batch, 2 batches per core on 8 cores.

Per-core pipeline (per batch):
  - SWDGE cast-load x -> SBUF as f32r (rounded fp32, full-rate matmul dtype)
  - PE transpose-mode: xf^T chunks (n on partitions) -> PSUM -> ACT copy -> SBUF
  - MM1 (f32r): energy[i, :] accumulated over 32 k-chunks into 4 PSUM banks
  - softmax: DVE row-min, ACT exp(min - e) with row-sum accumulation,
    DVE reciprocal, scale by gamma/Z
  - PE transpose att -> att^T
  - MM2 (f32r): att^T.T @ xf accumulated over 4 j-chunks, DVE adds residual x
  - HWDGE store
"""

import sys

if "/opt/trn_rl_repo" not in sys.path:
    sys.path.insert(0, "/opt/trn_rl_repo")

from contextlib import ExitStack

import numpy as np

import concourse.bass as bass
import concourse.tile as tile
from concourse import bacc, mybir
from concourse.bass_utils import run_bass_kernel_spmd
from concourse.masks import make_identity

N_CORES = 8
B, C, H, W = 16, 512, 64, 64
N = H * W                    # 4096
BPC = B // N_CORES           # batches per core = 2
CT = C // 128                # 4 c-tiles
KT = N // 128                # 32 k-chunks (transposed layout)
NCH = N // 512               # 8 moving chunks for MM2

F32 = mybir.dt.float32
F32R = mybir.dt.float32r


def _build_nc(reps=1):
    nc = bacc.Bacc("TRN2", target_bir_lowering=False, debug=False,
                   num_devices=N_CORES)
    x_d = nc.dram_tensor("x", [BPC, C, N], F32, kind="ExternalInput").ap()
    g_d = nc.dram_tensor("gamma", [1], F32, kind="ExternalInput").ap()
    o_d = nc.dram_tensor("out", [BPC, C, N], F32, kind="ExternalOutput").ap()

    with tile.TileContext(nc) as tc, ExitStack() as ctx:
        xf_pool = ctx.enter_context(tc.tile_pool(name="xf", bufs=BPC * CT * 4))
        xfT_pool = ctx.enter_context(tc.tile_pool(name="xfT", bufs=6))
        s_pool = ctx.enter_context(tc.tile_pool(name="s", bufs=CT))
        att_pool = ctx.enter_context(tc.tile_pool(name="att", bufs=CT))
        attT_pool = ctx.enter_context(tc.tile_pool(name="attT", bufs=CT))
        out_pool = ctx.enter_context(tc.tile_pool(name="outp", bufs=2))
        stat_pool = ctx.enter_context(tc.tile_pool(name="stat", bufs=4 * CT))
        one_pool = ctx.enter_context(tc.tile_pool(name="one", bufs=1))
        pT = ctx.enter_context(tc.tile_pool(name="pT", bufs=2, space="PSUM"))
        pE = ctx.enter_context(tc.tile_pool(name="pE", bufs=CT, space="PSUM"))
        pO = ctx.enter_context(tc.tile_pool(name="pO", bufs=2, space="PSUM"))

        # identity for PE transpose-mode (f32r so dtypes match the data)
        ident_f = one_pool.tile([128, 128], F32, tag="idf")
        make_identity(nc, ident_f[:])
        ident = one_pool.tile([128, 128], F32R, tag="idr")
        nc.vector.tensor_copy(ident[:], ident_f[:])

        # broadcast gamma to all 128 partitions via K=1 matmul with ones
        g_sb = one_pool.tile([1, 1], F32, tag="gsb")
        nc.sync.dma_start(g_sb[:], g_d.rearrange("(a b) -> a b", a=1))
        ones = one_pool.tile([1, 128], F32, tag="ones")
        nc.vector.memset(ones[:], 1.0)
        pG = pT.tile([128, 1], F32, tag="pt", name="pG")
        nc.tensor.matmul(pG[:], ones[:], g_sb[:], start=True, stop=True)
        g_bc = one_pool.tile([128, 1], F32, tag="gbc")
        nc.vector.tensor_copy(g_bc[:], pG[:])

        loop_ctx = tc.For_i(0, reps, 1) if reps > 1 else None
        if loop_ctx is not None:
            ctx.enter_context(loop_ctx)
        NQ = 4            # load chunks per c-tile (so compute starts early)
        QW = N // NQ      # 1024 columns per chunk
        for b in range(BPC):
            # ---- load x (cast f32 -> f32r during DMA) ----
            # chunked column-major so the first transposes only wait for
            # ~1/NQ of the input
            xf = [[None] * NQ for _ in range(CT)]
            for q in range(NQ):
                for ct in range(CT):
                    t = xf_pool.tile([128, QW], F32R, tag="xf",
                                     name=f"xf_{b}_{ct}_{q}")
                    nc.gpsimd.dma_start(
                        t[:],
                        x_d[b, ct * 128:(ct + 1) * 128, q * QW:(q + 1) * QW],
                    )
                    xf[ct][q] = t

            # ---- transpose to (n, c) layout + Gram matmuls ----
            e_ps = [
                pE.tile([128, C], F32, tag="pe", name=f"pe_{b}_{i}")
                for i in range(CT)
            ]
            for k in range(KT):
                tp = pT.tile([128, C], F32R, tag="pt")
                kq, kr = divmod(k, KT // NQ)
                for ct in range(CT):
                    nc.tensor.transpose(
                        tp[:, ct * 128:(ct + 1) * 128],
                        xf[ct][kq][:, kr * 128:(kr + 1) * 128],
                        ident[:],
                    )
                xT = xfT_pool.tile([128, C], F32R, tag="xT")
                nc.scalar.copy(xT[:], tp[:])
                for it in range(CT):
                    nc.tensor.matmul(
                        e_ps[it][:],
                        xT[:, it * 128:(it + 1) * 128],
                        xT[:],
                        start=(k == 0),
                        stop=(k == KT - 1),
                    )

            # ---- softmax(-energy) rows, scaled by gamma ----
            att = []
            for it in range(CT):
                m = stat_pool.tile([128, 1], F32, tag="m")
                nc.vector.tensor_reduce(
                    m[:], e_ps[it][:], axis=mybir.AxisListType.X,
                    op=mybir.AluOpType.min,
                )
                s = s_pool.tile([128, C], F32, tag="s")
                z = stat_pool.tile([128, 1], F32, tag="z")
                nc.scalar.activation(
                    s[:], e_ps[it][:], mybir.ActivationFunctionType.Exp,
                    bias=m[:], scale=-1.0, accum_out=z[:],
                )
                rz = stat_pool.tile([128, 1], F32, tag="rz")
                nc.vector.reciprocal(rz[:], z[:])
                g = stat_pool.tile([128, 1], F32, tag="g")
                nc.vector.tensor_mul(g[:], rz[:], g_bc[:])
                a = att_pool.tile([128, C], F32R, tag="a")
                nc.vector.tensor_scalar_mul(a[:], s[:], g[:])
                att.append(a)

            # ---- transpose attention ----
            attT = []
            for jt in range(CT):
                tp = pT.tile([128, C], F32R, tag="pt")
                for it in range(CT):
                    nc.tensor.transpose(
                        tp[:, it * 128:(it + 1) * 128],
                        att[it][:, jt * 128:(jt + 1) * 128],
                        ident[:],
                    )
                aT = attT_pool.tile([128, C], F32R, tag="aT")
                nc.vector.tensor_copy(aT[:], tp[:])
                attT.append(aT)

            # ---- out = att @ xf + x ----
            for it in range(CT):
                o = out_pool.tile([128, N], F32, tag="o")
                for nch in range(NCH):
                    po = pO.tile([128, 512], F32, tag="po")
                    for jt in range(CT):
                        nc.tensor.matmul(
                            po[:],
                            attT[jt][:, it * 128:(it + 1) * 128],
                            xf[jt][nch // 2][:, (nch % 2) * 512:
                                             (nch % 2 + 1) * 512],
                            start=(jt == 0),
                            stop=(jt == CT - 1),
                        )
                    nc.vector.tensor_add(
                        o[:, nch * 512:(nch + 1) * 512], po[:],
                        xf[it][nch // 2][:, (nch % 2) * 512:
                                         (nch % 2 + 1) * 512],
                    )
                for h in range(2):
                    nc.sync.dma_start(
                        o_d[b, it * 128:(it + 1) * 128,
                            h * (N // 2):(h + 1) * (N // 2)],
                        o[:, h * (N // 2):(h + 1) * (N // 2)],
                    )

    nc.compile()
    return nc


_RUNNER = None


def _build_runner(nc=None):
    """Compile once; return a callable (xf_full, gamma) -> out_full.

    Mirrors concourse.bass2jax.run_bass_via_pjrt but caches the jitted
    shard_map executable so repeated kernel() calls don't re-lower, and
    keeps the output-seed zero buffers resident on device.
    """
    import jax
    from jax.sharding import Mesh, NamedSharding, PartitionSpec
    from jax.experimental.shard_map import shard_map

    from concourse import bass2jax, mybir as _mybir
    from concourse.bass2jax import _bass_exec_p, partition_id_tensor

    if nc is None:
        nc = _build_nc()
    bass2jax.install_neuronx_cc_hook()

    partition_name = (
        nc.partition_id_tensor.name if nc.partition_id_tensor else None
    )
    in_names, out_names, out_avals, zero_shapes = [], [], [], []
    for alloc in nc.m.functions[0].allocations:
        if not isinstance(alloc, _mybir.MemoryLocationSet):
            continue
        name = alloc.memorylocations[0].name
        if alloc.kind == "ExternalInput":
            if name != partition_name:
                in_names.append(name)
        elif alloc.kind == "ExternalOutput":
            shape = tuple(alloc.tensor_shape)
            dtype = _mybir.dt.np(alloc.dtype)
            out_names.append(name)
            out_avals.append(jax.core.ShapedArray(shape, dtype))
            zero_shapes.append((shape, dtype))
    n_params = len(in_names)
    all_names = list(in_names) + list(out_names)
    if partition_name is not None:
        all_names.append(partition_name)
    donate = tuple(range(n_params, n_params + len(out_names)))

    def _body(*args):
        operands = list(args)
        if partition_name is not None:
            operands.append(partition_id_tensor())
        return tuple(
            _bass_exec_p.bind(
                *operands,
                out_avals=tuple(out_avals),
                in_names=tuple(all_names),
                out_names=tuple(out_names),
                lowering_input_output_aliases=(),
                sim_require_finite=True,
                sim_require_nnan=True,
                nc=nc,
            )
        )

    devices = jax.devices()[:N_CORES]
    mesh = Mesh(np.asarray(devices), ("core",))
    n_in = n_params + len(out_names)
    sharded = jax.jit(
        shard_map(
            _body,
            mesh=mesh,
            in_specs=(PartitionSpec("core"),) * n_in,
            out_specs=(PartitionSpec("core"),) * len(out_names),
            check_rep=False,
        ),
        keep_unused=True,
    )

    # in_names order is discovered from allocations; map our two inputs
    assert set(in_names) == {"x", "gamma"}, in_names

    # output-seed buffers created on device once (kernel writes out fully)
    sh = NamedSharding(mesh, PartitionSpec("core"))
    zeros_dev = [
        jax.jit(
            lambda s=s, d=d: jax.numpy.zeros((N_CORES * s[0],) + s[1:], d),
            out_shardings=sh,
        )()
        for s, d in zero_shapes
    ]
    jax.block_until_ready(zeros_dev)

    def run(xf_full, gamma):
        per_in = {
            "x": xf_full,  # (16, 512, 4096) == concat of per-core (2, 512, 4096)
            "gamma": np.ascontiguousarray(
                np.broadcast_to(np.asarray(gamma, np.float32).reshape(1),
                                (N_CORES,))
            ),
        }
        concat_in = [per_in[name] for name in in_names]
        out_arrs = sharded(*concat_in, *zeros_dev)
        return np.asarray(out_arrs[out_names.index("out")])

    run.sharded = sharded
    run.zeros_dev = zeros_dev
    run.in_names = in_names
    run.out_names = out_names
    run.mesh = mesh
    return run


def _get_runner():
    global _RUNNER
    if _RUNNER is None:
        _RUNNER = _build_runner()
    return _RUNNER


def kernel(x, gamma):
    assert x.shape == (B, C, H, W)
    run = _get_runner()
    xf = np.ascontiguousarray(np.asarray(x, np.float32).reshape(B, C, N))
    g = np.asarray(gamma, np.float32)
    out = run(xf, g)
    return out.reshape(B, C, H, W).astype(np.float32, copy=False)
